# revision 1
# baseline (speedup 1.0000x reference)
"""Self-contained Trainium2 Bass kernel for the dense transformer block.

Head-parallel sharding: each of the 8 cores computes Q/K/V + causal
attention for 2 of the 16 heads across BOTH batch elements, then an 8-rank
AllToAll per batch redistributes attention outputs token-major so each core
owns one 512-token chunk for the (full-width) output projection, residual,
LayerNorms and FFN. Batch-0's exchange overlaps batch-1's QKV+attention.
"""
import sys as _sys
if "/opt/trn_rl_repo" not in _sys.path:
    _sys.path.insert(0, "/opt/trn_rl_repo")

import numpy as np
import ml_dtypes

import concourse.bass as bass
import concourse.tile as tile
from concourse import bacc, mybir

F32 = mybir.dt.float32
BF16 = mybir.dt.bfloat16
AF = mybir.ActivationFunctionType
ALU = mybir.AluOpType

B, T, C, H, HS, FF = 2, 2048, 1024, 16, 64, 4096
TL = 512               # tokens per core (one chunk)
NCT = C // 128         # 8 feature tiles
NFT = FF // 128        # 32 ff tiles
NTC = B * T // TL      # 8 global token chunks
EPS = 1e-5
N_CORES = 8
VW = HS + 1            # 65: [v | ones] block per head
GB = 130               # a2a block rows: 128 attn feats + 2 denominators
SC = 0.125             # 1/sqrt(HS)


def build_program(dbg=False):
    nc = bacc.Bacc("TRN2", target_bir_lowering=False, debug=False,
                   enable_asserts=False, num_devices=N_CORES)

    xb_d = nc.dram_tensor("xb", (C, B * T), BF16, kind="ExternalInput").ap()
    xloc_d = nc.dram_tensor("xloc", (C, TL), F32, kind="ExternalInput").ap()
    wqkv_d = nc.dram_tensor("wqkv", (C, 384), BF16, kind="ExternalInput").ap()
    wproj_d = nc.dram_tensor("wproj", (C, C), BF16, kind="ExternalInput").ap()
    w1_d = nc.dram_tensor("w1", (C, FF), BF16, kind="ExternalInput").ap()
    w2_d = nc.dram_tensor("w2", (FF, C), BF16, kind="ExternalInput").ap()
    bproj_d = nc.dram_tensor("bproj", (C,), F32, kind="ExternalInput").ap()
    b1_d = nc.dram_tensor("b1", (FF,), F32, kind="ExternalInput").ap()
    b2_d = nc.dram_tensor("b2", (C,), F32, kind="ExternalInput").ap()
    ln1g_d = nc.dram_tensor("ln1g", (C,), F32, kind="ExternalInput").ap()
    ln1b_d = nc.dram_tensor("ln1b", (C,), F32, kind="ExternalInput").ap()
    ln2g_d = nc.dram_tensor("ln2g", (C,), F32, kind="ExternalInput").ap()
    ln2b_d = nc.dram_tensor("ln2b", (C,), F32, kind="ExternalInput").ap()
    out_d = nc.dram_tensor("out", (C, TL), BF16,
                       kind="ExternalOutput").ap()
    dbg_d = {}
    if dbg:
        for name, shape, dt in [
                ("dbg_q", (128, B * T), BF16), ("dbg_k", (128, B * T), BF16),
                ("dbg_v", (128, 2 * VW), BF16), ("dbg_st", (VW, TL), BF16),
                ("dbg_den", (16, TL), F32), ("dbg_attn", (128, TL), BF16),
                ("dbg_r1", (128, TL), F32), ("dbg_x2f", (128, TL), F32)]:
            dbg_d[name] = nc.dram_tensor(name, shape, dt,
                                         kind="ExternalOutput").ap()

    with tile.TileContext(nc) as tc:
        _emit(tc, xb_d, xloc_d, wqkv_d, wproj_d, w1_d, w2_d, bproj_d, b1_d,
              b2_d, ln1g_d, ln1b_d, ln2g_d, ln2b_d, out_d, dbg_d)

    nc.compile()
    return nc


def _emit(tc, xb_d, xloc_d, wqkv_d, wproj_d, w1_d, w2_d, bproj_d, b1_d,
          b2_d, ln1g_d, ln1b_d, ln2g_d, ln2b_d, out_d, dbg_d={}):
    nc = tc.nc
    dmaq = [nc.sync, nc.scalar, nc.gpsimd]      # hw DMA-capable queues

    def dump(name, ap):
        if name in dbg_d:
            nc.sync.dma_start(dbg_d[name][:], ap)

    # ---------------- constants / small inputs ----------------
    const = tc.alloc_tile_pool(name="const", bufs=1)

    bproj_sb = const.tile([128, NCT], F32, tag="bproj")
    nc.scalar.dma_start(bproj_sb[:], bproj_d.rearrange("(a p) -> p a", p=128))
    b1_sb = const.tile([128, NFT], F32, tag="b1")
    nc.scalar.dma_start(b1_sb[:], b1_d.rearrange("(a p) -> p a", p=128))
    b2_sb = const.tile([128, NCT], F32, tag="b2")
    nc.scalar.dma_start(b2_sb[:], b2_d.rearrange("(a p) -> p a", p=128))
    ln1g_sb = const.tile([128, NCT], F32, tag="ln1g")
    nc.scalar.dma_start(ln1g_sb[:], ln1g_d.rearrange("(a p) -> p a", p=128))
    ln1b_sb = const.tile([128, NCT], F32, tag="ln1b")
    nc.scalar.dma_start(ln1b_sb[:], ln1b_d.rearrange("(a p) -> p a", p=128))
    ln2g_sb = const.tile([128, NCT], F32, tag="ln2g")
    nc.scalar.dma_start(ln2g_sb[:], ln2g_d.rearrange("(a p) -> p a", p=128))
    ln2b_sb = const.tile([128, NCT], F32, tag="ln2b")
    nc.scalar.dma_start(ln2b_sb[:], ln2b_d.rearrange("(a p) -> p a", p=128))

    ones_f32 = const.tile([128, 1], F32, tag="ones_f32")       # LN col-sum
    nc.gpsimd.memset(ones_f32[:], 1.0)
    eps_sb = const.tile([1, 1], F32, tag="eps")
    nc.gpsimd.memset(eps_sb[:], EPS)
    ones_big = const.tile([128, TL], BF16, tag="ones_big")     # mask source
    nc.gpsimd.memset(ones_big[:], 1.0)
    zero_sb = const.tile([128, TL], BF16, tag="zero_sb")       # a2a padding
    nc.gpsimd.memset(zero_sb[:], 0.0)
    ident = const.tile([128, 128], BF16, tag="ident")          # PE transpose
    nc.gpsimd.affine_select(ident[:], ones_big[:, 0:128], pattern=[[1, 128]],
                            compare_op=ALU.is_equal, fill=0.0, base=0,
                            channel_multiplier=-1)
    onesc = const.tile([1, 128], F32, tag="onesc")   # row-broadcast lhsT
    nc.gpsimd.memset(onesc[:], 1.0)
    onesc_g[0] = onesc
    # sel[s][r, p] = 1 iff p // 64 == r - 2s: one matmul broadcasts the
    # 16 per-head reciprocal rows into the [128, TL] normalizer for tile s
    ones16 = const.tile([16, 128], BF16, tag="ones16")
    nc.gpsimd.memset(ones16[:], 1.0)
    sel = []
    for s in range(N_CORES):
        st_ = const.tile([16, 128], BF16, tag=f"selt{s}", name=f"selt{s}")
        nc.gpsimd.affine_select(st_[:], ones16[:], pattern=[[1, 128]],
                                compare_op=ALU.is_ge, fill=0.0, base=128 * s,
                                channel_multiplier=-64)
        se = const.tile([16, 128], BF16, tag=f"sel{s}", name=f"sel{s}")
        nc.gpsimd.affine_select(se[:], st_[:], pattern=[[-1, 128]],
                                compare_op=ALU.is_ge, fill=0.0,
                                base=63 - 128 * s, channel_multiplier=64)
        sel.append(se)

    # causal masks for diagonal k-tiles: mask_i[p, t] = 1 iff i*128 + p <= t
    masks = []
    for i in range(4):
        m = const.tile([128, TL], BF16, tag=f"mask{i}", name=f"mask{i}")
        nc.gpsimd.affine_select(
            m[:], ones_big[:], pattern=[[1, TL]],
            compare_op=ALU.is_ge, fill=0.0, base=-i * 128,
            channel_multiplier=-1)
        masks.append(m)

    # ---------------- activation storage ----------------
    xloc_pool = tc.alloc_tile_pool(name="xloc_pool", bufs=1)   # ..P3
    xloc = [xloc_pool.tile([128, TL], F32, tag=f"xloc{i}", name=f"xloc{i}")
            for i in range(NCT)]

    kqv_pool = tc.alloc_tile_pool(name="kqv_pool", bufs=1)     # P1..P2
    q_sb = kqv_pool.tile([128, B * T], BF16, tag="q", name="q")
    k_sb = kqv_pool.tile([128, B * T], BF16, tag="k", name="k")
    v_sb = [kqv_pool.tile([128, 2 * VW], BF16, tag=f"v{j}", name=f"v{j}")
            for j in range(2 * (T // 128))]                    # 32 k-tiles
    wqkv_sb = [kqv_pool.tile([128, 384], BF16, tag=f"wqkv{i}",
                             name=f"wqkv{i}") for i in range(NCT)]
    vf_sb = kqv_pool.tile([128, T], BF16, tag="vf", name="vf")

    # right-side pools: alloc order sets the LIFO release order:
    # xbf (top, released after P1b1) -> raw -> wproj -> p4w (bottom)
    p4w = tc.alloc_tile_pool(name="p4w_pool", bufs=1, side="right")
    wproj_pool = tc.alloc_tile_pool(name="wproj", bufs=1, side="right")
    wproj_sb = [wproj_pool.tile([128, C], BF16, tag=f"wp{i}", name=f"wp{i}")
                for i in range(NCT)]
    raw_pool = tc.alloc_tile_pool(name="raw_pool", bufs=1, side="right")
    attn_raw = [[raw_pool.tile([128, TL], BF16, tag=f"ar{b}_{s}",
                               name=f"ar{b}_{s}") for s in range(N_CORES)]
                for b in range(B)]
    den_raw = [raw_pool.tile([16, TL], BF16, tag=f"dr{b}", name=f"dr{b}")
               for b in range(B)]
    xbf_pool = tc.alloc_tile_pool(name="xbf_pool", bufs=1, side="right")
    xbf = [xbf_pool.tile([128, B * T], BF16, tag=f"xbf{i}", name=f"xbf{i}")
           for i in range(NCT)]

    # input DMA, priority order, striped across the three hw queues
    for i in range(NCT):
        nc.sync.dma_start(wqkv_sb[i][:], wqkv_d[i * 128:(i + 1) * 128, :])
    qi = 0
    for tcol in range(4):                       # 1024-col stripes, b0 first
        for i in range(NCT):
            dmaq[qi % 3].dma_start(
                xbf[i][:, tcol * 1024:(tcol + 1) * 1024],
                xb_d[i * 128:(i + 1) * 128, tcol * 1024:(tcol + 1) * 1024])
            qi += 1
    for i in range(NCT):
        dmaq[i % 3].dma_start(wproj_sb[i][:],
                              wproj_d[i * 128:(i + 1) * 128, :])
    for i in range(NCT):
        dmaq[i % 3].dma_start(xloc[i][:], xloc_d[i * 128:(i + 1) * 128, :])

    # a2a DRAM bounce buffers (one pair per batch), zero-padded in the
    # half that belongs to the other batch
    dram = tc.alloc_tile_pool(name="dram", bufs=1, space="DRAM")
    a2a_in = [dram.tile([N_CORES * GB, TL], BF16, tag=f"a2a_in{b}",
                        name=f"a2a_in{b}") for b in range(B)]
    a2a_out = [dram.tile([N_CORES * GB, TL], BF16, tag=f"a2a_out{b}",
                         name=f"a2a_out{b}") for b in range(B)]
    for b in range(B):
        for g in range(4 * (1 - b), 4 * (2 - b)):
            nc.sync.dma_start(a2a_in[b][g * GB:g * GB + 128, :], zero_sb[:])
            nc.sync.dma_start(a2a_in[b][g * GB + 128:(g + 1) * GB, :],
                              zero_sb[0:2, :])

    # ---------------- P1+P2 per batch, A2A overlapped ----------------
    with tc.tile_pool(name="p12sbuf", bufs=1) as p2sb:
        warm = p2sb.tile([1, 1], F32, tag="warm", name="warm")
        nc.scalar.activation(warm[:], eps_sb[:], AF.Exp)

        for b in range(B):
            # ---- P1(b): QKV for this batch's 4 token chunks ----
            with tc.tile_pool(name=f"p1ps{b}", bufs=1, space="PSUM") as p1ps:
                for tch in range(4 * b, 4 * b + 4):
                    t0 = tch * TL
                    for cols, dst in ((slice(128, 256), k_sb),
                                      (slice(0, 128), q_sb),
                                      (slice(256, 384), vf_sb)):
                        ps = p1ps.tile([128, TL], F32, tag="p1", bufs=3,
                                       name="p1")
                        for c in range(NCT):
                            nc.tensor.matmul(ps[:], wqkv_sb[c][:, cols],
                                             xbf[c][:, t0:t0 + TL],
                                             start=(c == 0),
                                             stop=(c == NCT - 1))
                        if dst is vf_sb:
                            lt0 = (tch - 4 * b) * TL
                            nc.vector.tensor_copy(dst[:, lt0:lt0 + TL],
                                                  ps[:])
                        else:
                            nc.vector.tensor_copy(dst[:, t0:t0 + TL], ps[:])
                    for kk in range(4):
                        kt = 4 * tch + kk
                        ps2 = p1ps.tile([128, 128], BF16, tag="p1v", bufs=2,
                                        name="p1v")
                        lk = kt - 16 * b
                        nc.tensor.transpose(
                            ps2[:], vf_sb[:, lk * 128:(lk + 1) * 128],
                            ident[:])
                        vj = v_sb[kt].rearrange("p (h w) -> p h w", w=VW)
                        nc.gpsimd.memset(vj[:, :, HS:VW], 1.0)
                        nc.vector.tensor_copy(
                            vj[:, :, 0:HS],
                            ps2[:].rearrange("p (h w) -> p h w", w=HS))

            # ---- P2(b): causal attention for the 2 local heads ----
            p2ps = tc.alloc_tile_pool(name=f"p2ps{b}", bufs=1, space="PSUM")
            for j in range(4):                    # local q-chunks of 512
                g = 4 * b + j                     # global chunk id
                q0 = b * T + j * TL
                nkt = 4 * j + 4                   # causal k-tiles
                avs = [p2ps.tile([VW, TL], F32, tag="av", bufs=4, name="av")
                       for _ in range(2)]
                for kt in range(nkt):
                    sc = p2ps.tile([128, 2 * TL], F32, tag="sc", bufs=2,
                                   name="sc")
                    for hh, po in ((0, 0), (1, 64)):
                        nc.tensor.matmul(
                            sc[:, hh * TL:(hh + 1) * TL],
                            k_sb[po:po + HS,
                                 b * T + kt * 128:b * T + (kt + 1) * 128],
                            q_sb[po:po + HS, q0:q0 + TL],
                            start=True, stop=True, tile_position=(po, 0))
                    probs = p2sb.tile([128, 2 * TL], BF16, tag="probs",
                                      bufs=6, name="probs")
                    nc.scalar.activation(probs[:], sc[:], AF.Exp, scale=SC)
                    if kt >= nkt - 4:
                        mm_ = masks[kt - (nkt - 4)]
                        for hh in range(2):
                            nc.vector.tensor_mul(
                                probs[:, hh * TL:(hh + 1) * TL],
                                probs[:, hh * TL:(hh + 1) * TL], mm_[:])
                    for hh in range(2):
                        nc.tensor.matmul(
                            avs[hh][:],
                            v_sb[b * 16 + kt][:, hh * VW:(hh + 1) * VW],
                            probs[:, hh * TL:(hh + 1) * TL],
                            start=(kt == 0), stop=(kt == nkt - 1))
                # stage unnormalized attn + denominators into the A2A src
                sts = []
                for hh in range(2):
                    sth = p2sb.tile([VW, TL], BF16, tag=f"st{hh}", bufs=1,
                                    name=f"st{hh}")
                    nc.vector.tensor_copy(sth[:], avs[hh][:])
                    sts.append(sth)
                    nc.gpsimd.dma_start(
                        a2a_in[b][g * GB + hh * HS:g * GB + (hh + 1) * HS,
                                  :],
                        sth[0:HS, :])
                    nc.gpsimd.dma_start(
                        a2a_in[b][g * GB + 128 + hh:g * GB + 129 + hh, :],
                        sth[HS:VW, :])
                if g == 0:
                    dump("dbg_st", sts[0][:])
            # batch b staged: fire its A2A; prefetch results on sync queue
            nc.gpsimd.collective_compute(
                "AllToAll", mybir.AluOpType.bypass,
                replica_groups=[list(range(N_CORES))],
                ins=[a2a_in[b][:].opt()], outs=[a2a_out[b][:].opt()])
            for s in range(N_CORES):
                nc.sync.dma_start(attn_raw[b][s][:],
                                  a2a_out[b][s * GB:s * GB + 128, :])
                nc.sync.dma_start(den_raw[b][2 * s:2 * s + 2, :],
                                  a2a_out[b][s * GB + 128:(s + 1) * GB, :])
            p2ps.release()

    xbf_pool.release()
    kqv_pool.release()

    # ---------------- P3: normalize + projection + residual + LN1 -------
    with tc.tile_pool(name="p3sbuf", bufs=1) as p3sb, \
         tc.tile_pool(name="p3psum", bufs=1, space="PSUM") as p3ps:
        attn_n = [p3sb.tile([128, TL], BF16, tag=f"an{s}", name=f"an{s}")
                  for s in range(N_CORES)]
        for s in range(N_CORES):
            nc.vector.tensor_add(attn_n[s][:], attn_raw[0][s][:],
                                 attn_raw[1][s][:])
        denf = p3sb.tile([16, TL], F32, tag="denf", name="denf")
        nc.vector.tensor_add(denf[:], den_raw[0][:], den_raw[1][:])
        dump("dbg_den", denf[:])
        raw_pool.release()
        rcp16f = p3sb.tile([16, TL], F32, tag="rcp16f", name="rcp16f")
        nc.vector.reciprocal(rcp16f[:], denf[:])
        rcp16 = p3sb.tile([16, TL], BF16, tag="rcp16", name="rcp16")
        nc.vector.tensor_copy(rcp16[:], rcp16f[:])
        for s in range(N_CORES):
            bcp = p3ps.tile([128, TL], F32, tag="bcp", bufs=2, name="bcp")
            nc.tensor.matmul(bcp[:], sel[s][:], rcp16[:],
                             start=True, stop=True)
            nc.vector.tensor_mul(attn_n[s][:], attn_n[s][:], bcp[:])
        dump("dbg_attn", attn_n[0][:])

        resid1 = [p3sb.tile([128, TL], F32, tag=f"r1_{i}", name=f"r1_{i}")
                  for i in range(NCT)]
        for e in range(NCT):
            ps = p3ps.tile([128, TL], F32, tag="pr", bufs=2, name="pr")
            for s in range(NCT):
                nc.tensor.matmul(
                    ps[:], wproj_sb[s][:, e * 128:(e + 1) * 128],
                    attn_n[s][:], start=(s == 0), stop=(s == NCT - 1))
            sa = p3sb.tile([128, TL], F32, tag="sa", bufs=3, name="sa")
            nc.scalar.activation(sa[:], ps[:], AF.Identity,
                                 bias=bproj_sb[:, e:e + 1])
            nc.vector.tensor_add(resid1[e][:], sa[:], xloc[e][:])
        dump("dbg_r1", resid1[0][:])
        wproj_pool.release()

        x2_pool = tc.alloc_tile_pool(name="x2_pool", bufs=1, side="right")
        w1late = tc.alloc_tile_pool(name="w1late", bufs=1, side="right")
        w1late_g[0] = w1late
        x2f = [x2_pool.tile([128, TL], F32, tag=f"x2f{i}", name=f"x2f{i}")
               for i in range(NCT)]
        x2b = [x2_pool.tile([128, TL], BF16, tag=f"x2b{i}", name=f"x2b{i}")
               for i in range(NCT)]

        # prefetch the first w1 quarter + w2 eighth during LN1
        w1q = [w1late.tile([128, 1024], BF16, tag=f"w1q{i}", bufs=2,
                           name=f"w1q{i}") for i in range(NCT)]
        for i in range(NCT):
            dmaq[i % 3].dma_start(w1q[i][:],
                                  w1_d[i * 128:(i + 1) * 128, 0:1024])
        w2e0 = [p4w.tile([128, C], BF16, tag=f"w2e{i}", bufs=2,
                         name=f"w2e{i}") for i in range(4)]
        for i in range(4):
            dmaq[i % 3].dma_start(w2e0[i][:], w2_d[i * 128:(i + 1) * 128, :])
        p4w_state[0] = (w1q, w2e0)

        _layernorm(tc, p3sb, p3ps, resid1, ln1g_sb, ln1b_sb, ones_f32,
                   eps_sb, out_f32=x2f, out_bf16=x2b)
        dump("dbg_x2f", x2f[0][:])
    xloc_pool.release()

    # ---------------- P4: FFN ----------------
    r2_pool = tc.alloc_tile_pool(name="r2_pool", bufs=1)
    resid2 = [r2_pool.tile([128, TL], F32, tag=f"r2_{i}", name=f"r2_{i}")
              for i in range(NCT)]

    statps = tc.alloc_tile_pool(name="statps", bufs=1, space="PSUM")
    mu2_ps = statps.tile([1, TL], F32, tag="mu2", name="mu2")
    sq2_ps = statps.tile([1, TL], F32, tag="sq2", name="sq2")
    with tc.tile_pool(name="p4sbuf", bufs=1) as p4sb, \
         tc.tile_pool(name="p4psum", bufs=1, space="PSUM") as p4ps:
        for qt in range(4):
            f0 = qt * 8
            if qt == 0:
                w1q, w2e = p4w_state[0]
            else:
                w1q = [w1late_g[0].tile([128, 1024], BF16, tag=f"w1q{i}",
                                        bufs=2, name=f"w1q{i}")
                       for i in range(NCT)]
                for i in range(NCT):
                    dmaq[i % 3].dma_start(
                        w1q[i][:], w1_d[i * 128:(i + 1) * 128,
                                        f0 * 128:(f0 + 8) * 128])
            h_sb = [p4sb.tile([128, TL], BF16, tag=f"h{i}", bufs=3,
                              name=f"h{i}") for i in range(8)]
            for fi in range(8):
                f = f0 + fi
                ps = p4ps.tile([128, TL], F32, tag="h1", bufs=3, name="h1")
                for c in range(NCT):
                    nc.tensor.matmul(
                        ps[:], w1q[c][:, fi * 128:(fi + 1) * 128], x2b[c][:],
                        start=(c == 0), stop=(c == NCT - 1))
                nc.vector.tensor_scalar(
                    h_sb[fi][:], ps[:], b1_sb[:, f:f + 1], 0.0,
                    op0=ALU.add, op1=ALU.max)
            for eighth in range(2):
                if not (qt == 0 and eighth == 0):
                    w2e = [p4w.tile([128, C], BF16, tag=f"w2e{i}", bufs=2,
                                    name=f"w2e{i}") for i in range(4)]
                    for i in range(4):
                        f = f0 + eighth * 4 + i
                        dmaq[i % 3].dma_start(
                            w2e[i][:], w2_d[f * 128:(f + 1) * 128, :])
                for e in range(NCT):
                    ps = p4ps.tile([128, TL], F32, tag="ff", bufs=3,
                                   name="ff")
                    for i in range(4):
                        nc.tensor.matmul(
                            ps[:], w2e[i][:, e * 128:(e + 1) * 128],
                            h_sb[eighth * 4 + i][:],
                            start=(i == 0), stop=(i == 3))
                    if qt == 0 and eighth == 0:
                        tmp = p4sb.tile([128, TL], F32, tag="ft", bufs=3,
                                        name="ft")
                        nc.scalar.activation(tmp[:], ps[:], AF.Identity,
                                             bias=b2_sb[:, e:e + 1])
                        nc.vector.tensor_add(resid2[e][:], tmp[:], x2f[e][:])
                    else:
                        nc.vector.tensor_add(resid2[e][:], resid2[e][:],
                                             ps[:])
                    if qt == 3 and eighth == 1:
                        # LN2 statistics interleave with the last FFN2 pass
                        nc.tensor.matmul(mu2_ps[:], ones_f32[:],
                                         resid2[e][:], start=(e == 0),
                                         stop=(e == NCT - 1))
                        sq2t = p4sb.tile([128, TL], F32, tag="sq2t", bufs=2,
                                         name="sq2t")
                        nc.scalar.square(sq2t[:], resid2[e][:])
                        nc.tensor.matmul(sq2_ps[:], ones_f32[:], sq2t[:],
                                         start=(e == 0),
                                         stop=(e == NCT - 1))

    w1late_g[0].release()
    x2_pool.release()
    p4w.release()

    # ---------------- LN2 + output ----------------
    with tc.tile_pool(name="p5sbuf", bufs=1) as p5sb, \
         tc.tile_pool(name="p5psum", bufs=1, space="PSUM") as p5ps:
        of = [p5sb.tile([128, TL], BF16, tag=f"of{i}", name=f"of{i}")
              for i in range(NCT)]
        _layernorm(tc, p5sb, p5ps, resid2, ln2g_sb, ln2b_sb, ones_f32,
                   eps_sb, out_f32=of, out_bf16=None,
                   stats=(mu2_ps, sq2_ps),
                   out_dma=[(out_d[i * 128:(i + 1) * 128, :], dmaq[i % 3])
                            for i in range(NCT)])

    statps.release()
    r2_pool.release()
    dram.release()
    const.release()


onesc_g = [None]
p4w_state = [None]
w1late_g = [None]


def _layernorm(tc, sb_pool, ps_pool, resid, g_sb, b_sb, ones, eps_sb,
               out_f32, out_bf16, stats=None, out_dma=None):
    """Feature-major LayerNorm over the partition (feature) axis."""
    nc = tc.nc
    if stats is None:
        mu_ps = ps_pool.tile([1, TL], F32, tag="mu", bufs=1, name="mu")
        sq_ps = ps_pool.tile([1, TL], F32, tag="sq", bufs=1, name="sq")
        for i in range(NCT):
            nc.tensor.matmul(mu_ps[:], ones[:], resid[i][:],
                             start=(i == 0), stop=(i == NCT - 1))
        for i in range(NCT):
            sqt_i = sb_pool.tile([128, TL], F32, tag="sqt", bufs=2,
                                 name="sqt")
            nc.scalar.square(sqt_i[:], resid[i][:])
            nc.tensor.matmul(sq_ps[:], ones[:], sqt_i[:],
                             start=(i == 0), stop=(i == NCT - 1))
    else:
        mu_ps, sq_ps = stats

    mu = sb_pool.tile([1, TL], F32, tag="lnmu", bufs=1, name="lnmu")
    nc.vector.tensor_scalar_mul(mu[:], mu_ps[:], 1.0 / C)
    ms = sb_pool.tile([1, TL], F32, tag="lnms", bufs=1, name="lnms")
    nc.vector.tensor_scalar_mul(ms[:], sq_ps[:], 1.0 / C)
    mu2 = sb_pool.tile([1, TL], F32, tag="lnmu2", bufs=1, name="lnmu2")
    nc.vector.tensor_mul(mu2[:], mu[:], mu[:])
    var = sb_pool.tile([1, TL], F32, tag="lnvar", bufs=1, name="lnvar")
    nc.vector.tensor_sub(var[:], ms[:], mu2[:])
    sd = sb_pool.tile([1, TL], F32, tag="lnsd", bufs=1, name="lnsd")
    nc.scalar.activation(sd[:], var[:], AF.Sqrt, bias=eps_sb[:])
    rstd = sb_pool.tile([1, TL], F32, tag="lnrstd", bufs=1, name="lnrstd")
    nc.vector.reciprocal(rstd[:], sd[:])

    mu_bc = ps_pool.tile([128, TL], F32, tag="lnmubc", bufs=1,
                         name="lnmubc")
    nc.tensor.matmul(mu_bc[:], onesc_g[0][:], mu[:], start=True, stop=True)
    rs_bc = ps_pool.tile([128, TL], F32, tag="lnrsbc", bufs=1,
                         name="lnrsbc")
    nc.tensor.matmul(rs_bc[:], onesc_g[0][:], rstd[:], start=True,
                     stop=True)

    for i in range(NCT):
        nc.vector.tensor_sub(resid[i][:], resid[i][:], mu_bc[:])
        nc.vector.tensor_mul(resid[i][:], resid[i][:], rs_bc[:])
        nc.vector.tensor_scalar(out_f32[i][:], resid[i][:], g_sb[:, i:i + 1],
                                b_sb[:, i:i + 1], op0=ALU.mult, op1=ALU.add)
        if out_bf16 is not None:
            nc.vector.tensor_copy(out_bf16[i][:], out_f32[i][:])
        if out_dma is not None:
            dst, q = out_dma[i]
            q.dma_start(dst, out_f32[i][:])


# ---------------- host side ----------------

def host_prepare(x, wq, wk, wv, wproj, bproj, ln1_g, ln1_b, w1, b1, w2, b2,
                 ln2_g, ln2_b):
    bf = ml_dtypes.bfloat16
    xT = np.concatenate([np.ascontiguousarray(x[b].T) for b in range(B)],
                        axis=1)                       # [C, B*T] fp32
    xb = xT.astype(bf)
    shared = {
        "xb": xb,
        "wproj": wproj.astype(bf),
        "w1": w1.astype(bf),
        "w2": w2.astype(bf),
        "bproj": bproj.astype(np.float32),
        "b1": b1.astype(np.float32),
        "b2": b2.astype(np.float32),
        "ln1g": ln1_g.astype(np.float32),
        "ln1b": ln1_b.astype(np.float32),
        "ln2g": ln2_g.astype(np.float32),
        "ln2b": ln2_b.astype(np.float32),
    }
    in_maps = []
    for core in range(N_CORES):
        h0 = 2 * core
        wqkv = np.concatenate(
            [wq[h0], wq[h0 + 1], wk[h0], wk[h0 + 1], wv[h0], wv[h0 + 1]],
            axis=1).astype(bf)                        # [C, 384]
        b, d = divmod(core, 4)
        xloc = np.ascontiguousarray(
            xT[:, b * T + d * TL: b * T + (d + 1) * TL]).astype(np.float32)
        in_maps.append({"wqkv": wqkv, "xloc": xloc, **shared})
    return in_maps


def host_finalize(results):
    out = np.empty((B, T, C), np.float32)
    for core in range(N_CORES):
        b, d = divmod(core, 4)
        out[b, d * TL:(d + 1) * TL, :] = \
            results[core]["out"].T.astype(np.float32)
    return out


# ---------------- top-level entry ----------------
from concourse.bass_utils import run_bass_kernel_spmd as _run_spmd

_nc_cache = None


def _program():
    global _nc_cache
    if _nc_cache is None:
        _nc_cache = build_program()
    return _nc_cache


def run(inputs, trace=False):
    nc = _program()
    in_maps = host_prepare(**inputs)
    res = _run_spmd(nc, in_maps, core_ids=list(range(N_CORES)), trace=trace)
    return host_finalize(res.results), res


def kernel(**inputs):
    out, _ = run(inputs, trace=False)
    return out

